# revision 5
# baseline (speedup 1.0000x reference)
"""GCN (GCNConv + ReLU + Linear) Trainium2 kernel, 8-core SPMD.

Strategy v2 (per core, owning a 12500-node dst range):
  - Host precomputes x_scaled = x * rsqrt(deg) (node-level) into a bf16
    table [100352, 128] (14 features + zero pad), split in 4 quartiles
    for int16 gather indices.
  - Host packs edges (self-loops excluded) by (bank=dst//512, src-quartile,
    window=(dst%512)//128), padded to 128-edge batches (pad idx=0, slot
    sentinel so the one-hot zeroes pads).
  - Device per (bank, quartile) cell: dma_gather bf16 rows, build 128-wide
    one-hot via iota-compare (bf16), scatter-accumulate with bf16 PE
    matmuls into PSUM [14, 512].
  - Dense tail per bank: (agg + x_scaled_own) * rsqrt(deg_dst) @ W1 + b1
    -> relu -> @ W2 + b2.  (The x_scaled_own term is the self-loop.)
"""
import numpy as np
import ml_dtypes

N = 100000
NE = 3200000
F = 14
H = 64
NC = 8
QS = 25088          # quartile rows (4*QS = 100352 padded table rows)
NPAD = 4 * QS
OWN = N // NC       # 12500
W = 128             # dst window width
NWIN = 4            # windows per psum bank (512 cols)
NBANK = 25          # 25 * 512 = 12800 padded own-dst
DCORE = NBANK * 512


def _ranks(keys_sorted):
    """rank of each element within its (already grouped) key run."""
    n = len(keys_sorted)
    if n == 0:
        return np.zeros(0, dtype=np.int64)
    change = np.ones(n, dtype=bool)
    change[1:] = keys_sorted[1:] != keys_sorted[:-1]
    run_start = np.maximum.accumulate(np.where(change, np.arange(n), 0))
    return np.arange(n) - run_start


def _host_pack(edge_index):
    """Partition/sort/pad edges; returns per-core index streams + metadata."""
    src = edge_index[0].astype(np.int64)
    dst = edge_index[1].astype(np.int64)
    deg = (np.bincount(dst, minlength=N) + 1).astype(np.float32)  # + self-loop

    owner = dst // OWN
    per_core = []
    for c in range(NC):
        m = owner == c
        s, d = src[m], dst[m] - c * OWN
        bank = d >> 9
        q = s // QS
        wl = (d >> 7) & 3
        slot = d & 127
        wq = (bank * 4 + q) * 4 + wl      # (bank, quartile, window) id
        order = np.argsort(wq, kind="stable")
        per_core.append((s[order] % QS, slot[order], wq[order]))

    nwq = NBANK * 4 * NWIN
    B = 1
    for s_, sl_, wq_ in per_core:
        cnt = np.bincount(wq_, minlength=nwq)
        B = max(B, int(np.ceil(cnt.max() / 128)))
    CELL_E = NWIN * B * 128
    L = NBANK * 4 * CELL_E

    idx_streams, slot_streams = [], []
    for s_, sl_, wq_ in per_core:
        idx = np.zeros(L, dtype=np.int16)           # pad: row 0 (valid)
        slo = np.full(L, 1000.0, dtype=np.float32)  # pad: no one-hot match
        r = _ranks(wq_)
        # batch-interleaved placement: cell-relative batch = (r//128)*4 + wl
        pos = (wq_ >> 2) * CELL_E + ((r >> 7) * 4 + (wq_ & 3)) * 128 + (r & 127)
        idx[pos] = s_.astype(np.int16)
        slo[pos] = sl_.astype(np.float32)
        idx_streams.append(idx)
        slot_streams.append(slo)
    return deg, idx_streams, slot_streams, B, L


def _build_program(B):
    import concourse.bass as bass
    import concourse.mybir as mybir
    from concourse import bacc
    from concourse.tile import TileContext

    CELL_E = NWIN * B * 128           # edges per (bank, quartile) gather cell
    G = CELL_E // 128                 # batches per cell

    nc = bacc.Bacc("TRN2", target_bir_lowering=False, debug=False, num_devices=NC)
    dt = mybir.dt

    x128q = [
        nc.dram_tensor(f"x128q{q}", [QS, 128], dt.bfloat16, kind="ExternalInput")
        for q in range(4)
    ]
    idx16 = nc.dram_tensor("idx16", [128, (NBANK * 4 * CELL_E) // 16], dt.int16,
                           kind="ExternalInput")
    slots = nc.dram_tensor("slots", [128, (NBANK * 4 * CELL_E) // 128], dt.float32,
                           kind="ExternalInput")
    iota128 = nc.dram_tensor("iota128", [128, W], dt.float32, kind="ExternalInput")
    degown = nc.dram_tensor("degown", [1, DCORE], dt.float32, kind="ExternalInput")
    xownt = nc.dram_tensor("xownt", [F, DCORE], dt.float32, kind="ExternalInput")
    w1 = nc.dram_tensor("w1", [F, H], dt.float32, kind="ExternalInput")
    b1 = nc.dram_tensor("b1", [H, 1], dt.float32, kind="ExternalInput")
    w2 = nc.dram_tensor("w2", [H, 1], dt.float32, kind="ExternalInput")
    b2 = nc.dram_tensor("b2", [1, 1], dt.float32, kind="ExternalInput")
    yout = nc.dram_tensor("yout", [1, DCORE], dt.float32, kind="ExternalOutput")

    with TileContext(nc) as tc:
        with (
            tc.tile_pool(name="persist", bufs=1) as pp,
            tc.tile_pool(name="gather", bufs=2) as gp,
            tc.tile_pool(name="work", bufs=2) as wp,
            tc.tile_pool(name="psum", bufs=2, space="PSUM") as psp,
            tc.tile_pool(name="psum_t", bufs=2, space="PSUM") as pst,
        ):
            # ---- persistent small tensors ----
            iota_sb = pp.tile([128, W], dt.float32)
            nc.sync.dma_start(iota_sb[:], iota128[:])
            w1_sb = pp.tile([F, H], dt.float32)
            nc.sync.dma_start(w1_sb[:], w1[:])
            b1_sb = pp.tile([H, 1], dt.float32)
            nc.sync.dma_start(b1_sb[:], b1[:])
            w2_sb = pp.tile([H, 1], dt.float32)
            nc.sync.dma_start(w2_sb[:], w2[:])
            b2_sb = pp.tile([1, 1], dt.float32)
            nc.sync.dma_start(b2_sb[:], b2[:])
            xownt_sb = pp.tile([F, DCORE], dt.float32)
            nc.sync.dma_start(xownt_sb[:], xownt[:])
            y_sb = pp.tile([1, DCORE], dt.float32)
            ones_f = pp.tile([1, F], dt.float32)
            nc.vector.memset(ones_f[:], 1.0)

            # ---- main loop: 25 banks x 4 quartiles ----
            for bank in range(NBANK):
                pbank = psp.tile([F, 512], dt.float32)
                for q in range(4):
                    cell = bank * 4 + q
                    idx_sb = gp.tile([128, CELL_E // 16], dt.int16, tag="idx")
                    nc.sync.dma_start(
                        idx_sb[:],
                        idx16[:, cell * (CELL_E // 16):(cell + 1) * (CELL_E // 16)],
                    )
                    gath = gp.tile([128, G * 128], dt.bfloat16, tag="gath")
                    nc.gpsimd.dma_gather(
                        out_ap=gath[:].rearrange("p (g e) -> p g e", e=128),
                        in_ap=x128q[q][:],
                        idxs_ap=idx_sb[:],
                        num_idxs=CELL_E,
                        num_idxs_reg=CELL_E,
                        elem_size=128,
                        single_packet=False,
                    )
                    # one-hot = (slot == iota), bf16
                    slot_sb = gp.tile([128, G], dt.float32, tag="slot")
                    nc.sync.dma_start(
                        slot_sb[:], slots[:, cell * G:(cell + 1) * G]
                    )
                    oh = wp.tile([128, G * W], dt.bfloat16, tag="oh")
                    nc.vector.tensor_tensor(
                        out=oh[:].rearrange("p (g w) -> p g w", w=W),
                        in0=slot_sb[:].unsqueeze(2).to_broadcast([128, G, W]),
                        in1=iota_sb[:].unsqueeze(1).to_broadcast([128, G, W]),
                        op=mybir.AluOpType.is_equal,
                    )
                    # scatter matmuls (batch j serves window j%4)
                    for j in range(G):
                        wl = j % 4
                        nc.tensor.matmul(
                            out=pbank[:, wl * W:(wl + 1) * W],
                            lhsT=gath[:, j * 128:j * 128 + F],
                            rhs=oh[:, j * W:(j + 1) * W],
                            start=(q == 0 and j == 0),
                            stop=(q == 3 and j == G - 1),
                        )
                # per-bank dense tail:
                # (pbank + xown_scaled) * rsqrt(deg_dst) -> W1 -> relu -> W2
                db = wp.tile([1, 512], dt.float32, tag="db")
                nc.sync.dma_start(db[:], degown[:, bank * 512:(bank + 1) * 512])
                dbs = wp.tile([1, 512], dt.float32, tag="dbs")
                nc.scalar.activation(
                    dbs[:], db[:], mybir.ActivationFunctionType.Sqrt
                )
                nc.vector.reciprocal(dbs[:], dbs[:])
                pdv = pst.tile([F, 512], dt.float32, tag="pdv")
                nc.tensor.matmul(
                    out=pdv[:], lhsT=ones_f[:], rhs=dbs[:],
                    start=True, stop=True,
                )
                aggb = wp.tile([F, 512], dt.float32, tag="aggb")
                nc.vector.tensor_tensor(
                    out=aggb[:], in0=pbank[:],
                    in1=xownt_sb[:, bank * 512:(bank + 1) * 512],
                    op=mybir.AluOpType.add,
                )
                nc.vector.tensor_tensor(
                    out=aggb[:], in0=aggb[:], in1=pdv[:],
                    op=mybir.AluOpType.mult,
                )
                ph = pst.tile([H, 512], dt.float32, tag="ph")
                nc.tensor.matmul(
                    out=ph[:], lhsT=w1_sb[:], rhs=aggb[:],
                    start=True, stop=True,
                )
                hb = wp.tile([H, 512], dt.float32, tag="hb")
                nc.scalar.activation(
                    hb[:], ph[:],
                    mybir.ActivationFunctionType.Relu,
                    bias=b1_sb[:],
                )
                py = pst.tile([1, 512], dt.float32, tag="py")
                nc.tensor.matmul(
                    out=py[:], lhsT=w2_sb[:], rhs=hb[:],
                    start=True, stop=True,
                )
                nc.vector.tensor_scalar(
                    out=y_sb[:, bank * 512:(bank + 1) * 512],
                    in0=py[:], scalar1=b2_sb[:], scalar2=None,
                    op0=mybir.AluOpType.add,
                )
            nc.sync.dma_start(yout[:], y_sb[:])

    nc.compile()
    return nc


_CACHE = {}


def kernel(x, edge_index, W1, b1, W2, b2, _want_results_obj=False):
    from concourse import bass_utils

    x = np.asarray(x, dtype=np.float32)
    edge_index = np.asarray(edge_index)
    deg, idx_streams, slot_streams, B, L = _host_pack(edge_index)

    if B not in _CACHE:
        _CACHE[B] = _build_program(B)
    nc = _CACHE[B]

    dinv = 1.0 / np.sqrt(deg)
    xs = x * dinv[:, None]                       # x_scaled, fp32 [N, 14]
    xt = np.zeros((NPAD, 128), dtype=np.float32)
    xt[:N, :F] = xs
    xq = xt.astype(ml_dtypes.bfloat16)
    iota = np.broadcast_to(np.arange(W, dtype=np.float32), (128, W)).copy()

    in_maps = []
    for c in range(NC):
        idx = idx_streams[c]
        idx16 = np.tile(np.ascontiguousarray(idx.reshape(-1, 16).T), (8, 1))
        slots = np.ascontiguousarray(slot_streams[c].reshape(-1, 128).T)
        degown = np.ones((1, DCORE), dtype=np.float32)
        degown[0, :OWN] = deg[c * OWN:(c + 1) * OWN]
        xownt = np.zeros((F, DCORE), dtype=np.float32)
        xownt[:, :OWN] = xs[c * OWN:(c + 1) * OWN].T
        in_maps.append({
            **{f"x128q{q}": np.ascontiguousarray(xq[q * QS:(q + 1) * QS])
               for q in range(4)},
            "idx16": np.ascontiguousarray(idx16),
            "slots": slots,
            "iota128": iota,
            "degown": degown,
            "xownt": xownt,
            "w1": np.asarray(W1, dtype=np.float32),
            "b1": np.asarray(b1, dtype=np.float32).reshape(H, 1),
            "w2": np.asarray(W2, dtype=np.float32),
            "b2": np.asarray(b2, dtype=np.float32).reshape(1, 1),
        })

    res = bass_utils.run_bass_kernel_spmd(nc, in_maps, core_ids=list(range(NC)))
    y = np.concatenate([res.results[c]["yout"][0, :OWN] for c in range(NC)])
    out = y.reshape(N, 1).astype(np.float32)
    if _want_results_obj:
        return out, res
    return out
